# revision 30
# baseline (speedup 1.0000x reference)
"""Trainium2 Bass kernel for nn_Attention (LayerNorm + MHA + out-proj).

Sharding: 8 cores = 4 batch elements x 2 query-halves. Each core receives its
batch element's full token sequence (rolled so its 1024 query rows are first),
computes LayerNorm + K/V projections over all 2048 tokens, Q projection and
attention for its 1024 query rows, and the output projection. No collectives.

Layout strategy (single SPMD Bass program, feature-on-partition style):
  x [2048,512] --LN--> xc(bf16) --PE transpose--> xnT [4][128d, 2048tok] bf16
  Kt/Qt per head-pair: [128(2x64), tok] bf16  (W chunk stationary, xnT moving)
  V:                   [128tok, 8x65] bf16    (xnT stationary, W moving;
                                               65th col = ones for row-sums)
  St per (pair,qt,kc): [128k, 2x512q] PSUM    (Kt stationary, Qt moving;
                                               the two heads of a pair run as
                                               concurrent 64-row PE tiles)
  A = exp(St/8)        ACT/DVE -> bf16 SBUF
  O^T per head:        [65, 512q] PSUM accum  (V stationary, A moving)
  normalize: row 64 -> reciprocal_approx_fast -> gpsimd bcast -> DVE mult
  out^T = W_out^T @ O^T (bf16) + b_out; host transposes back.

vs. the previous revision (213.5us -> ~202us on HW):
  - ln_gamma/ln_beta are folded into W_qkv on the host (W' = diag(gamma) @ W,
    b' = beta @ W applied as per-partition evac biases; the V-section bias is
    folded through W_out into b_out since softmax rows sum to 1), so the LN
    apply is a plain (x-mu)*rstd and rstd comes from one ACT op
    (Abs_reciprocal_sqrt).
  - one PSUM pool for the whole kernel (tags: 2x[128,1024] + 4x[128,512])
    instead of per-phase pools -- no pool-transition barriers.
  - per-token-tile pipeline in phase A (LN -> 4 transposes -> one DVE evac),
    V projection per tile (lag 2), K/Q projection units spread one-per-tile
    across the loop so their evacuations never burst on one engine.
  - PE warm-up transposes at kernel start and before the final out-proj keep
    the HAM activity monitor from dropping the PE clock to 1.2 GHz; the ones
    memset only touches the 65th columns so make_identity isn't blocked.
  - out-proj for q-tile 0 is emitted in the middle of q-tile 1's attention, so
    its wait on the softmax normalize never blocks the PE queue (the 3.7us
    qt-boundary bubbles previously re-throttled the PE clock for ~13us).
"""

import numpy as np
import ml_dtypes

import concourse.bass as bass
import concourse.tile as tile
from concourse import bacc, mybir
from concourse.bass_utils import run_bass_kernel_spmd
from concourse.masks import make_identity

F32 = mybir.dt.float32
BF16 = mybir.dt.bfloat16
I16 = mybir.dt.int16
ADD = mybir.AluOpType.add
MULT = mybir.AluOpType.mult

B, N, D = 4, 2048, 512
H, DH = 8, 64
NQ = N // 2          # query rows per core
SCALE = DH ** -0.5   # 0.125
NCORES = 8

# Schraudolph fast-exp on DVE for a subset of key chunks: bf16 bit pattern of
# exp(s*SCALE) ~= int16(round(A*s + B)); softmax numerator/denominator both use
# the same approximate weights, so the ratio error stays small.
SEXP_A = float(SCALE * 128 / np.log(2))
SEXP_B = float(128 * 127 - 4.5)
SEXP_KCS = frozenset((2, 5, 8, 11, 14))  # 5 of 16 chunks go to DVE

QT = NQ // 512       # 2 query tiles of 512
KC = N // 128        # 16 key chunks of 128
TT = N // 128        # 16 token tiles of 128
DC = D // 128        # 4 feature chunks of 128

DEBUG_TAPS = False   # adds DRAM dumps of intermediates (xnT, kt, qt, v)


def build_program(out_dtype=F32):
    nc = bacc.Bacc("TRN2", target_bir_lowering=False, debug=False)

    x_ap = nc.dram_tensor("x", [N, D], F32, kind="ExternalInput").ap()
    wqkv_ap = nc.dram_tensor("w_qkv", [D, 3 * D], BF16, kind="ExternalInput").ap()
    wout_ap = nc.dram_tensor("w_out", [D, D], BF16, kind="ExternalInput").ap()
    bout_ap = nc.dram_tensor("b_out", [D], F32, kind="ExternalInput").ap()
    bkq_ap = nc.dram_tensor("b_kq", [128, 8], F32, kind="ExternalInput").ap()
    y_ap = nc.dram_tensor("y_t", [D, NQ], BF16, kind="ExternalOutput").ap()
    taps = None
    if DEBUG_TAPS:
        taps = {
            "xnT": nc.dram_tensor("d_xnT", [128, DC, N], BF16, kind="ExternalOutput").ap(),
            "kt": nc.dram_tensor("d_kt", [128, 4, N], BF16, kind="ExternalOutput").ap(),
            "qt": nc.dram_tensor("d_qt", [128, 4, NQ], BF16, kind="ExternalOutput").ap(),
            "v": nc.dram_tensor("d_v", [128, TT, H * (DH + 1)], BF16, kind="ExternalOutput").ap(),
            "ot": nc.dram_tensor("d_ot", [128, 4, NQ], BF16, kind="ExternalOutput").ap(),
        }

    with tile.TileContext(nc) as tc:
        attention_kernel(tc, y_ap, x_ap, wqkv_ap, wout_ap, bout_ap, bkq_ap,
                         taps=taps)
    nc.compile()
    return nc


def attention_kernel(tc, y_ap, x_ap, wqkv_ap, wout_ap, bout_ap, bkq_ap,
                     taps=None):
    nc = tc.nc
    from contextlib import ExitStack

    with ExitStack() as ctx:
        persist = ctx.enter_context(tc.tile_pool(name="persist", bufs=1))
        work = ctx.enter_context(tc.tile_pool(name="work", bufs=3))
        psum = ctx.enter_context(tc.tile_pool(name="psum", bufs=1, space="PSUM"))

        def big_tile(name):
            return psum.tile([128, 1024], F32, tag="big", bufs=2, name=name)

        def sm_tile(name):
            return psum.tile([128, 512], F32, tag="sm", bufs=4, name=name)

        # ---- input DMAs, ordered for the phase-A pipeline: x tile 0 first
        # (smallest possible first-LN latency), weights interleaved so each
        # consumer's data lands before its matmuls reach the PE queue ----
        xts = persist.tile([128, TT, D], F32)
        xsrc = x_ap.rearrange("(g p) d -> p g d", p=128)
        wq_sb = persist.tile([128, DC, 3 * D], BF16)
        wsrc = wqkv_ap.rearrange("(c p) e -> p c e", p=128)
        for g in range(4):
            nc.sync.dma_start(xts[:, g, :], xsrc[:, g, :])
        nc.sync.dma_start(wq_sb[:, :, 2 * D:3 * D], wsrc[:, :, 2 * D:3 * D])  # V
        nc.sync.dma_start(wq_sb[:, :, D:2 * D], wsrc[:, :, D:2 * D])   # K
        for g in range(4, 8):
            nc.sync.dma_start(xts[:, g, :], xsrc[:, g, :])
        nc.sync.dma_start(wq_sb[:, :, 0:D], wsrc[:, :, 0:D])           # Q
        for g in range(8, 16):
            nc.sync.dma_start(xts[:, g, :], xsrc[:, g, :])

        bkq_sb = persist.tile([128, 8], F32)
        nc.sync.dma_start(bkq_sb, bkq_ap)
        bias_sb = persist.tile([128, DC], F32)
        nc.sync.dma_start(bias_sb, bout_ap.rearrange("(c p) -> p c", p=128))
        wo_sb = persist.tile([128, DC, D], BF16)
        nc.sync.dma_start(wo_sb, wout_ap.rearrange("(c p) e -> p c e", p=128))

        # ---- constants ----
        identity = persist.tile([128, 128], BF16)
        make_identity(nc, identity)
        eps_sb = persist.tile([128, 1], F32)
        nc.vector.memset(eps_sb, 1e-5)
        # PE warm-up: keep the HAM activity monitor busy until the first real
        # transpose arrives, so phase A runs at 2.4 GHz instead of 1.2
        warm = psum.tile([128, 512], BF16, tag="sm", bufs=4, name="warm")
        for w in range(12):
            nc.tensor.transpose(warm[:, 128 * (w % 4):128 * (w % 4) + 128],
                                identity, identity)

        # ---- persistent activations ----
        xnT = persist.tile([128, DC, N], BF16)        # [d-part, dchunk, tok]
        kt_all = persist.tile([128, 4, N], BF16)      # [2x64 head rows, pair, tok]
        qt_all = persist.tile([128, 4, NQ], BF16)     # [2x64 head rows, pair, qtok]
        v_all = persist.tile([128, TT, H * (DH + 1)], BF16)  # [tok, tt, 8x65]
        ot_all = persist.tile([128, 4, NQ], BF16)     # [2x64 inner rows, pair, qtok]

        # only the 65th column of each head needs the ones (row-sum trick);
        # a full-tile memset costs 7us on gpsimd and blocks make_identity
        nc.gpsimd.memset(
            v_all.rearrange("p t (h e) -> p t h e", e=DH + 1)[:, :, :, DH:DH + 1],
            1.0,
        )

        # ---- Phase A: per-token-tile pipeline (LN -> transpose -> evac),
        # V per tile (lagged 2), K/Q per 1024-token range ----
        def v_proj(tt):
            pv = sm_tile("pv")
            for dc in range(DC):
                nc.tensor.matmul(
                    pv,
                    lhsT=xnT[:, dc, tt * 128:(tt + 1) * 128],
                    rhs=wq_sb[:, dc, 2 * D:3 * D],
                    start=(dc == 0), stop=(dc == DC - 1),
                )
            nc.scalar.activation(
                out=v_all[:, tt, :].rearrange("p (h e) -> p h e", e=DH + 1)[:, :, 0:DH],
                in_=pv.rearrange("p (h d) -> p h d", d=DH),
                func=mybir.ActivationFunctionType.Identity, scale=1.0,
            )

        def kq_unit(sec, p, r, dst, bias_col, evac_dve=False):
            # one (pair, 512-token-range) projection: 4 dc-matmuls + evac
            pk = sm_tile("pk")
            for dc in range(DC):
                nc.tensor.matmul(
                    pk,
                    lhsT=wq_sb[:, dc, sec + p * 128:sec + (p + 1) * 128],
                    rhs=xnT[:, dc, r * 512:(r + 1) * 512],
                    start=(dc == 0), stop=(dc == DC - 1),
                )
            if evac_dve:
                nc.vector.tensor_scalar_add(
                    dst[:, p, r * 512:(r + 1) * 512], pk,
                    bkq_sb[:, bias_col + p:bias_col + p + 1],
                )
            else:
                nc.scalar.activation(
                    out=dst[:, p, r * 512:(r + 1) * 512], in_=pk,
                    func=mybir.ActivationFunctionType.Identity,
                    bias=bkq_sb[:, bias_col + p:bias_col + p + 1], scale=1.0,
                )

        # spread the K/Q units across the tile loop (each needs xnT for its
        # range only) so the ACT evacuations never burst and stall the ring
        KQ_SCHED = {
            4: [("K", 0, 0), ("K", 1, 0)], 5: [("K", 2, 0), ("K", 3, 0)],
            8: [("Q", 0, 0), ("K", 0, 1)], 9: [("Q", 1, 0), ("K", 1, 1)],
            10: [("Q", 2, 0), ("K", 2, 1)], 11: [("Q", 3, 0), ("K", 3, 1)],
            12: [("Q", 0, 1), ("K", 0, 2)], 13: [("Q", 1, 1), ("K", 1, 2)],
            14: [("Q", 2, 1), ("K", 2, 2)], 15: [("Q", 3, 1), ("K", 3, 2)],
        }

        for tt in range(TT):
            xt = xts[:, tt, :]
            from contextlib import nullcontext
            hint = (tc.tile_wait_until(0.0022 * tt) if 1 <= tt <= 5
                    else nullcontext())
            with hint:
                stats = work.tile([128, 6], F32, tag="stats", bufs=6, name="stats")
                nc.vector.bn_stats(out=stats, in_=xt)
            mv = work.tile([128, 2], F32, tag="mv", bufs=6, name="mv")
            nc.vector.bn_aggr(out=mv, in_=stats)
            rstd = work.tile([128, 1], F32, tag="rstd", bufs=6, name="rstd")
            nc.scalar.activation(
                out=rstd, in_=mv[:, 1:2],
                func=mybir.ActivationFunctionType.Abs_reciprocal_sqrt,
                bias=eps_sb, scale=1.0,
            )
            xc = work.tile([128, D], BF16, tag="xc", bufs=6, name="xc")
            if tt < 2:
                # first tiles are on the kernel's critical path: finish the LN
                # apply on ACT so a DVE queue full of bn_stats can't delay it
                negr = work.tile([128, 1], F32, tag="negr", bufs=2, name="negr")
                nc.vector.tensor_scalar(
                    out=negr, in0=mv[:, 0:1], scalar1=rstd, scalar2=-1.0,
                    op0=MULT, op1=MULT,
                )
                nc.scalar.activation(
                    out=xc, in_=xt,
                    func=mybir.ActivationFunctionType.Identity,
                    bias=negr, scale=rstd,
                )
            else:
                nc.vector.tensor_scalar(
                    out=xc, in0=xt, scalar1=mv[:, 0:1], scalar2=rstd,
                    op0=mybir.AluOpType.subtract, op1=MULT,
                )
            tpg = psum.tile([128, 512], BF16, tag="sm", bufs=4, name="tpg")
            for dc in range(DC):
                nc.tensor.transpose(
                    tpg[:, dc * 128:(dc + 1) * 128],
                    xc[:, dc * 128:(dc + 1) * 128], identity,
                )
            nc.vector.tensor_copy(
                xnT[:, :, tt * 128:(tt + 1) * 128],
                tpg.rearrange("p (c e) -> p c e", e=128),
            )
            if tt >= 1:
                v_proj(tt - 1)
            for kind, p, r in KQ_SCHED.get(tt, []):
                if kind == "K":
                    kq_unit(D, p, r, kt_all, 0)
                else:
                    kq_unit(0, p, r, qt_all, 4)
        v_proj(TT - 1)
        for p in range(4):
            kq_unit(D, p, 3, kt_all, 0, evac_dve=True)   # K range 3

        if taps is not None:
            nc.sync.dma_start(taps["xnT"], xnT)
            nc.sync.dma_start(taps["kt"], kt_all)
            nc.sync.dma_start(taps["qt"], qt_all)
            nc.sync.dma_start(taps["v"], v_all)

        # ---- Phase C: attention; out-proj for qt is emitted inside qt+1's
        # p-loop so its normalize-wait never blocks the PE queue ----
        def out_proj_unit(qt, dm, yf):
            yp = sm_tile("yp")
            for p in range(4):
                nc.tensor.matmul(
                    yp,
                    lhsT=wo_sb[:, p, dm * 128:(dm + 1) * 128],
                    rhs=ot_all[:, p, qt * 512:(qt + 1) * 512],
                    start=(p == 0), stop=(p == 3),
                )
            nc.scalar.activation(
                out=yf[:, dm, :], in_=yp,
                func=mybir.ActivationFunctionType.Identity,
                bias=bias_sb[:, dm:dm + 1], scale=1.0,
            )

        def out_proj(qt, yf=None):
            ydst = y_ap.rearrange("(c p) q -> p c q", p=128)
            if yf is None:
                yf = work.tile([128, DC, 512], BF16, tag="yf", bufs=2, name="yf")
                for dm in range(2):
                    out_proj_unit(qt, dm, yf)
                nc.sync.dma_start(
                    ydst[:, 0:2, qt * 512:(qt + 1) * 512], yf[:, 0:2, :])
                for dm in range(2, DC):
                    out_proj_unit(qt, dm, yf)
                nc.sync.dma_start(
                    ydst[:, 2:4, qt * 512:(qt + 1) * 512], yf[:, 2:4, :])
            else:
                nc.sync.dma_start(
                    ydst[:, :, qt * 512:(qt + 1) * 512], yf)

        for qt in range(QT):
            for p in range(4):
                oacc = [sm_tile(f"o{i}") for i in range(2)]
                ats = {}
                # 2 kc per iteration: [S,S, AV,AV,AV,AV] lets the full-array
                # AV ldweights chain-prefetch (only the first is exposed after
                # the row-tiled S pairs, which block full-array weight loads)
                for bi in range(KC // 2 + 1):
                    sts = []
                    if bi < KC // 2:
                        for kc in (2 * bi, 2 * bi + 1):
                            st = big_tile("st")
                            sts.append((kc, st))
                            for half in range(2):
                                nc.tensor.matmul(
                                    st[:, half * 512:(half + 1) * 512],
                                    lhsT=kt_all[64 * half:64 * half + 64, p,
                                                kc * 128:(kc + 1) * 128],
                                    rhs=qt_all[64 * half:64 * half + 64, p,
                                               qt * 512:(qt + 1) * 512],
                                    start=True, stop=True,
                                )
                    if bi >= 1:
                        for pkc in (2 * bi - 2, 2 * bi - 1):
                            pat = ats.pop(pkc)
                            for half in range(2):
                                h = 2 * p + half
                                nc.tensor.matmul(
                                    oacc[half][0:DH + 1, :],
                                    lhsT=v_all[:, pkc, h * (DH + 1):(h + 1) * (DH + 1)],
                                    rhs=pat[:, half * 512:(half + 1) * 512],
                                    start=(pkc == 0), stop=(pkc == KC - 1),
                                )
                    for kc, st in sts:
                        at = work.tile([128, 1024], BF16, tag="at", bufs=8, name="at")
                        if kc in SEXP_KCS:
                            nc.vector.tensor_scalar(
                                out=at.bitcast(I16), in0=st,
                                scalar1=SEXP_A, scalar2=SEXP_B,
                                op0=MULT, op1=ADD,
                            )
                        else:
                            nc.scalar.activation(
                                out=at, in_=st,
                                func=mybir.ActivationFunctionType.Exp, scale=SCALE,
                            )
                        ats[kc] = at
                for half in range(2):
                    o_acc = oacc[half]
                    s_sb = work.tile([1, 512], F32, tag="s_sb", bufs=4, name="s_sb")
                    nc.vector.tensor_copy(s_sb, o_acc[DH:DH + 1, :])
                    r_sb = work.tile([1, 512], F32, tag="r_sb", bufs=4, name="r_sb")
                    nc.vector.reciprocal_approx_fast(out=r_sb, in_=s_sb)
                    cb_sb = work.tile([DH, 512], F32, tag="cb", bufs=4, name="cb")
                    nc.gpsimd.partition_broadcast(cb_sb, r_sb)
                    nc.vector.tensor_tensor(
                        ot_all[64 * half:64 * half + 64, p,
                               qt * 512:(qt + 1) * 512],
                        o_acc[0:DH, :], cb_sb, MULT,
                    )
                if qt == 1:
                    if p == 0:
                        yf0 = work.tile([128, DC, 512], BF16, tag="yf", bufs=2,
                                        name="yf0")
                    out_proj_unit(0, p, yf0)
                    if p == 3:
                        out_proj(0, yf=yf0)
                if qt == 1 and p == 3:
                    # keep the PE (and its HAM clock) busy while the last
                    # softmax normalize chain runs, so the final out-proj
                    # executes at 2.4 GHz
                    tailw = big_tile("tailw").bitcast(BF16)
                    for w in range(18):
                        nc.tensor.transpose(
                            tailw[:, 128 * (w % 4):128 * (w % 4) + 128],
                            identity, identity,
                        )
        out_proj(1)
        if taps is not None:
            nc.sync.dma_start(taps["ot"], ot_all)


_CACHED_NC = None


def _get_program():
    global _CACHED_NC
    if _CACHED_NC is None:
        _CACHED_NC = build_program()
    return _CACHED_NC


def make_in_maps(x, ln_gamma, ln_beta, W_qkv, W_out, b_out):
    x = np.asarray(x, dtype=np.float32)
    gamma = np.asarray(ln_gamma, dtype=np.float32)
    beta = np.asarray(ln_beta, dtype=np.float32)
    wqkv_f = np.asarray(W_qkv, dtype=np.float32)
    # fold LN gamma/beta into the QKV projection: (z*g+b)@W = z@(g[:,None]*W) + b@W
    wqkv_folded = (gamma[:, None] * wqkv_f).astype(ml_dtypes.bfloat16)
    b_qkv = beta @ wqkv_f  # [3*D] f32
    b_kq = np.empty((128, 8), dtype=np.float32)
    for p in range(4):
        b_kq[:, p] = b_qkv[D + p * 128:D + (p + 1) * 128]      # K pairs
        b_kq[:, 4 + p] = b_qkv[p * 128:(p + 1) * 128]          # Q pairs
    wout_f = np.asarray(W_out, dtype=np.float32)
    wout_bf = wout_f.astype(ml_dtypes.bfloat16)
    # V bias: attn rows sum to 1, so (attn @ (V + 1 bv^T)) @ Wout
    # = attn @ V @ Wout + bv @ Wout -- fold into the output bias.
    bout = (np.asarray(b_out, dtype=np.float32)
            + b_qkv[2 * D:3 * D] @ wout_f).astype(np.float32)
    in_maps = []
    for c in range(NCORES):
        b, qh = c // 2, c % 2
        xb = np.roll(x[b], -NQ * qh, axis=0)  # query rows first
        in_maps.append({
            "x": np.ascontiguousarray(xb),
            "w_qkv": wqkv_folded,
            "w_out": wout_bf,
            "b_out": bout,
            "b_kq": b_kq,
        })
    return in_maps


def kernel(x, ln_gamma, ln_beta, W_qkv, W_out, b_out):
    nc = _get_program()
    in_maps = make_in_maps(x, ln_gamma, ln_beta, W_qkv, W_out, b_out)
    res = run_bass_kernel_spmd(nc, in_maps, core_ids=list(range(NCORES)))

    y = np.empty((B, N, D), dtype=np.float32)
    for c in range(NCORES):
        b, qh = c // 2, c % 2
        y[b, NQ * qh:NQ * (qh + 1), :] = res.results[c]["y_t"].astype(np.float32).T
    return y
